# revision 1
# baseline (speedup 1.0000x reference)
"""Trainium2 Bass kernel: cross-modal channel attention.

Math (per batch b), with G the static [L, S] linear-interp matrix:
    q    = img_feat[b] reshaped [C, S]            (C=768, S=1024, L=77, D=512)
    tp   = text_feat[b] @ W_txt                   [L, C]
    t    = tp^T @ G                               [C, S]   (never materialized)
    logits^T = t @ q^T = tp^T @ (G @ q^T)         [Cj, Ci]  -- factored via L
    E^T  = exp(logits^T * S^-0.5)                 [Cj, Ci]
    Z_i  = sum_j E^T[j, i]   (ones-column matmuls)
    outA = E @ t = (tp @ E^T)^T @ G               [Ci, S]   -- factored via L
    out  = q + (gamma / Z_i) * outA               [C, S]

Sharding: data-parallel over batch across 8 cores (4 batches/core);
W_txt, G, gamma replicated.  Matmuls run in float32r (TF32 PE fast path,
1 cycle/row); fp32r operands are produced by rounding PSUM->SBUF copies /
activations.  The residual q stays exact fp32.
"""

import sys

sys.path.insert(0, "/opt/trn_rl_repo")

from contextlib import ExitStack

import numpy as np

import concourse.bacc as bacc
import concourse.mybir as mybir
import concourse.tile as tile
from concourse.bass_utils import run_bass_kernel_spmd
from concourse.masks import make_identity

B, C, HH, WW = 32, 768, 32, 32
S = HH * WW
L, D = 77, 512
N_CORES = 8
B_CORE = B // N_CORES
P = 128
CT, ST, DT = C // P, S // P, D // P
F32 = mybir.dt.float32
F32R = mybir.dt.float32r
SCALE = float(S) ** -0.5
EXP = mybir.ActivationFunctionType.Exp
MULT = mybir.AluOpType.mult
ADD = mybir.AluOpType.add


def _round_tf32(x):
    """Round fp32 -> tf32-representable (10-bit mantissa, round-to-nearest-even)."""
    u = np.ascontiguousarray(x, dtype=np.float32).view(np.uint32)
    r = (u + np.uint32(0x0FFF) + ((u >> np.uint32(13)) & np.uint32(1))) & np.uint32(
        0xFFFFE000
    )
    return r.view(np.float32)


def _interp_matrix():
    """G[l, s] such that (tp^T @ G)[c, s] == linear_interp(tp^T, S)[c, s]."""
    src = np.clip(
        (np.arange(S, dtype=np.float32) + np.float32(0.5)) * np.float32(L / S)
        - np.float32(0.5),
        np.float32(0.0),
        np.float32(L - 1),
    )
    i0 = np.floor(src).astype(np.int32)
    i1 = np.minimum(i0 + 1, L - 1)
    w = (src - i0.astype(np.float32)).astype(np.float32)
    g = np.zeros((L, S), dtype=np.float32)
    g[i0, np.arange(S)] += np.float32(1.0) - w
    g[i1, np.arange(S)] += w
    return g


def _build():
    nc = bacc.Bacc("TRN2", target_bir_lowering=False, debug=False)
    img = nc.dram_tensor("img", [B_CORE, C, S], F32, kind="ExternalInput").ap()
    txt = nc.dram_tensor("txt", [B_CORE, L, D], F32, kind="ExternalInput").ap()
    wt = nc.dram_tensor("wt", [D, C], F32R, kind="ExternalInput").ap()
    g = nc.dram_tensor("g", [L, S], F32R, kind="ExternalInput").ap()
    gt = nc.dram_tensor("gt", [S, L], F32R, kind="ExternalInput").ap()
    gamma = nc.dram_tensor("gamma128", [P, 1], F32, kind="ExternalInput").ap()
    out = nc.dram_tensor("out", [B_CORE, C, S], F32, kind="ExternalOutput").ap()

    with ExitStack() as ctx:
        tc = ctx.enter_context(tile.TileContext(nc))
        consts = ctx.enter_context(tc.tile_pool(name="consts", bufs=1))
        q_pool = ctx.enter_context(tc.tile_pool(name="q", bufs=2))
        txt_pool = ctx.enter_context(tc.tile_pool(name="txtp", bufs=2))
        small = ctx.enter_context(tc.tile_pool(name="small", bufs=2))
        qtb_pool = ctx.enter_context(tc.tile_pool(name="qtb", bufs=3))
        et_pool = ctx.enter_context(tc.tile_pool(name="et", bufs=2))
        outp = ctx.enter_context(tc.tile_pool(name="outp", bufs=2))
        zp = ctx.enter_context(tc.tile_pool(name="zp", bufs=3))
        # PSUM: small pool 2x1 bank + big pool 2x3 banks = 8 banks total.
        ps_small = ctx.enter_context(tc.tile_pool(name="ps_s", bufs=2, space="PSUM"))
        ps_big = ctx.enter_context(tc.tile_pool(name="ps_b", bufs=2, space="PSUM"))

        w_sb = consts.tile([P, DT, C], F32R)
        nc.sync.dma_start(w_sb[:], wt.rearrange("(k p) c -> p k c", p=P))
        g_sb = consts.tile([P, S], F32R)
        nc.sync.dma_start(g_sb[0:L, :], g)
        gt_sb = consts.tile([P, ST, L], F32R)
        nc.sync.dma_start(gt_sb[:], gt.rearrange("(st p) l -> p st l", p=P))
        gamma_sb = consts.tile([P, 1], F32)
        nc.sync.dma_start(gamma_sb[:], gamma)
        ident = consts.tile([P, P], F32)
        make_identity(nc, ident[:])
        # f32r memset/affine_select fail codegen -> produce via rounding copies
        ident_r = consts.tile([P, P], F32R)
        nc.vector.tensor_copy(ident_r[:], ident[:])
        ones_f = consts.tile([P, 2], F32)
        nc.gpsimd.memset(ones_f[:], 1.0)
        ones_sb = consts.tile([P, 2], F32R)
        nc.vector.tensor_copy(ones_sb[:], ones_f[:])

        for b in range(B_CORE):
            q_sb = q_pool.tile([P, CT, S], F32, tag="q")
            nc.sync.dma_start(q_sb[:], img[b].rearrange("(ct p) s -> p ct s", p=P))
            txt_sb = txt_pool.tile([P, D], F32, tag="txt")
            nc.sync.dma_start(txt_sb[0:L, :], txt[b])

            # text^T [D, L] via fp32 PE transposes; rounding copy -> f32r
            ps_tt = ps_small.tile([P, DT, P], F32, tag="ps")
            for k in range(DT):
                nc.tensor.transpose(
                    ps_tt[:, k, 0:L],
                    txt_sb[0:L, k * P : (k + 1) * P],
                    ident[0:L, 0:L],
                )
            txtT_sb = small.tile([P, DT, P], F32R, tag="txtT")
            nc.vector.tensor_copy(txtT_sb[:, :, 0:L], ps_tt[:, :, 0:L])

            # tp = text @ W_txt  [L, C]
            tp_sb = small.tile([P, C], F32R, tag="tp")
            ps_a = ps_small.tile([P, 512], F32, tag="ps")
            for k in range(DT):
                nc.tensor.matmul(
                    ps_a[0:L, :],
                    txtT_sb[:, k, 0:L],
                    w_sb[:, k, 0:512],
                    start=(k == 0),
                    stop=(k == DT - 1),
                )
            nc.scalar.copy(tp_sb[0:L, 0:512], ps_a[0:L, :])
            ps_b2 = ps_small.tile([P, 512], F32, tag="ps")
            for k in range(DT):
                nc.tensor.matmul(
                    ps_b2[0:L, 0:256],
                    txtT_sb[:, k, 0:L],
                    w_sb[:, k, 512:768],
                    start=(k == 0),
                    stop=(k == DT - 1),
                )
            nc.scalar.copy(tp_sb[0:L, 512:768], ps_b2[0:L, 0:256])

            # tp^T [C, L] via f32r PE transposes of tp
            # tp^T via regular matmul against identity (fp32r dst must be even -> N=78)
            ps_tp = ps_small.tile([P, CT, 80], F32, tag="ps")
            for jt in range(CT):
                nc.tensor.matmul(
                    ps_tp[:, jt, 0 : L + 1],
                    tp_sb[0:L, jt * P : (jt + 1) * P],
                    ident_r[0:L, 0 : L + 1],
                    start=True,
                    stop=True,
                )
            tpT_sb = small.tile([P, CT, 80], F32R, tag="tpT")
            nc.vector.tensor_copy(tpT_sb[:, :, 0:L], ps_tp[:, :, 0:L])

            # q^T blocks (streamed) + GQT = G @ q^T  [L, C] accumulated over S
            ps_gqt = ps_big.tile([P, 1025], F32, tag="psb")
            for st in range(ST):
                ps1 = ps_small.tile([P, 512], F32, tag="ps")
                for ct in range(4):
                    nc.tensor.transpose(
                        ps1[:, ct * P : (ct + 1) * P],
                        q_sb[:, ct, st * P : (st + 1) * P],
                        ident[:],
                    )
                ps2 = ps_small.tile([P, 512], F32, tag="ps")
                for ct in range(4, 6):
                    nc.tensor.transpose(
                        ps2[:, (ct - 4) * P : (ct - 3) * P],
                        q_sb[:, ct, st * P : (st + 1) * P],
                        ident[:],
                    )
                qtb = qtb_pool.tile([P, C], F32R, tag="qtb")
                if st % 2 == 0:
                    nc.vector.tensor_copy(qtb[:, 0:512], ps1[:, :])
                    nc.vector.tensor_copy(qtb[:, 512:768], ps2[:, 0:256])
                else:
                    nc.scalar.copy(qtb[:, 0:512], ps1[:, :])
                    nc.scalar.copy(qtb[:, 512:768], ps2[:, 0:256])
                nc.tensor.matmul(
                    ps_gqt[0:L, 0:512],
                    gt_sb[:, st, :],
                    qtb[:, 0:512],
                    start=(st == 0),
                    stop=(st == ST - 1),
                )
                nc.tensor.matmul(
                    ps_gqt[0:L, 512:768],
                    gt_sb[:, st, :],
                    qtb[:, 512:768],
                    start=(st == 0),
                    stop=(st == ST - 1),
                )
            gqt_sb = small.tile([P, C], F32R, tag="gqt")
            nc.scalar.copy(gqt_sb[0:L, :], ps_gqt[0:L, 0:C])

            # logits^T = tp^T @ GQT per j-tile, fused exp -> E^T (f32r)
            et_sb = et_pool.tile([P, CT, C], F32R, tag="et")
            for jt in range(CT):
                psl = ps_big.tile([P, 1025], F32, tag="psb")
                lhsT = tp_sb[0:L, jt * P : (jt + 1) * P]
                nc.tensor.matmul(
                    psl[:, 0:512], lhsT, gqt_sb[0:L, 0:512], start=True, stop=True
                )
                nc.tensor.matmul(
                    psl[:, 512:768], lhsT, gqt_sb[0:L, 512:768], start=True, stop=True
                )
                nc.scalar.activation(et_sb[:, jt, :], psl[:, 0:C], EXP, scale=SCALE)

            # ZT = tp @ E^T  [L, C] accumulated over j-tiles
            ps_z1 = ps_small.tile([P, 512], F32, tag="ps")
            ps_z2 = ps_small.tile([P, 512], F32, tag="ps")
            for jt in range(CT):
                nc.tensor.matmul(
                    ps_z1[0:L, :],
                    tpT_sb[:, jt, 0:L],
                    et_sb[:, jt, 0:512],
                    start=(jt == 0),
                    stop=(jt == CT - 1),
                )
                nc.tensor.matmul(
                    ps_z2[0:L, 0:256],
                    tpT_sb[:, jt, 0:L],
                    et_sb[:, jt, 512:768],
                    start=(jt == 0),
                    stop=(jt == CT - 1),
                )
            zt_sb = small.tile([P, C], F32R, tag="zt")
            nc.scalar.copy(zt_sb[0:L, 0:512], ps_z1[0:L, :])
            nc.scalar.copy(zt_sb[0:L, 512:768], ps_z2[0:L, 0:256])

            # outA = ZT^T @ G (+ Z_i from ones column), epilogue, store
            for it in range(CT):
                psa = ps_big.tile([P, 1026], F32, tag="psb")
                lhsT = zt_sb[0:L, it * P : (it + 1) * P]
                nc.tensor.matmul(
                    psa[:, 0:512], lhsT, g_sb[0:L, 0:512], start=True, stop=True
                )
                nc.tensor.matmul(
                    psa[:, 512:1024], lhsT, g_sb[0:L, 512:1024], start=True, stop=True
                )
                for jt in range(CT):
                    nc.tensor.matmul(
                        psa[:, 1024:1026],
                        et_sb[:, jt, it * P : (it + 1) * P],
                        ones_sb[:],
                        start=(jt == 0),
                        stop=(jt == CT - 1),
                    )
                rz = zp.tile([P, 1], F32, tag="rz")
                nc.vector.reciprocal(rz[:], psa[:, 1024:1025])
                gz = zp.tile([P, 1], F32, tag="gz")
                nc.vector.tensor_scalar_mul(gz[:], rz[:], gamma_sb[:])
                o_sb = outp.tile([P, S], F32, tag="o")
                nc.vector.scalar_tensor_tensor(
                    o_sb[:], psa[:, 0:1024], gz[:], q_sb[:, it, :], op0=MULT, op1=ADD
                )
                nc.sync.dma_start(
                    out[b].rearrange("(ct p) s -> ct p s", p=P)[it], o_sb[:]
                )

    nc.compile()
    return nc


_NC = None


def _get_nc():
    global _NC
    if _NC is None:
        _NC = _build()
    return _NC


def _in_maps(img_feat, text_feat, W_txt, gamma):
    img = np.ascontiguousarray(img_feat.reshape(B, C, S), dtype=np.float32)
    txt = np.ascontiguousarray(text_feat, dtype=np.float32)
    wt = _round_tf32(np.ascontiguousarray(W_txt, dtype=np.float32))
    g = _round_tf32(_interp_matrix())
    gt = np.ascontiguousarray(g.T)
    gamma128 = np.full((P, 1), np.float32(gamma.reshape(-1)[0]), dtype=np.float32)
    maps = []
    for m in range(N_CORES):
        sl = slice(m * B_CORE, (m + 1) * B_CORE)
        maps.append(
            {
                "img": np.ascontiguousarray(img[sl]),
                "txt": np.ascontiguousarray(txt[sl]),
                "wt": wt,
                "g": g,
                "gt": gt,
                "gamma128": gamma128,
            }
        )
    return maps


def _run(in_maps, **kwargs):
    nc = _get_nc()
    return run_bass_kernel_spmd(nc, in_maps, core_ids=list(range(N_CORES)), **kwargs)


def kernel(img_feat, text_feat, W_txt, gamma):
    res = _run(_in_maps(img_feat, text_feat, W_txt, gamma))
    full = np.concatenate([res.results[m]["out"] for m in range(N_CORES)], axis=0)
    return full.reshape(B, C, HH, WW).astype(np.float32)

